# revision 24
# baseline (speedup 1.0000x reference)
"""Multi-head attention (B=2, S=2048, D=1024, H=16) on 8 NeuronCores.

Sharding: core c -> (batch b = c // 4, head-group g = c % 4, 4 heads each).
Each core computes its 4 heads' attention for its batch plus the partial
output projection (ctx_shard @ WO_shard.T).T; the host sums the 4 partials
per batch, adds the bias, and patches fully-masked query rows (where the
reference's softmax degenerates to uniform attention).

v2 schedule: the exp stream on the scalar engine is the pacer (~80us of
activation work), so the kernel is organized as one continuous attention
stream that starts as early as possible and hides everything else inside it:
  - Strips run interleaved (hp,qt) = (0,0),(1,0),(0,1),(1,1),... so each
    qt's normalization + output projection pipeline into later strips.
  - Only Q/K s-tile 0 and V chunks 0-3 are projected up front; all other
    projection matmuls are emitted as "filler" between attention chunks,
    sized to the per-chunk exp-minus-PE bubble, keeping the PE dense (HAM
    stays warm) without starving the scores->exp chain.
  - Scalar engine does exp only during the stream; V-projection copy+mask
    is one vector op per chunk; the V ones-columns are filled by a single
    broadcast DMA of the bf16 padding mask.
  - Softmax normalization is deferred: V carries a ones column so P@V also
    accumulates row sums L[q]; per strip, L rows are DMA-packed into a
    [16,64] tile (fast reciprocal), bounced through DRAM for a partition
    broadcast, and divided into ctx while later strips run.
"""

import os
import sys

import numpy as np

sys.path.insert(0, "/opt/trn_rl_repo")
os.environ.setdefault("MYCRO_LOCAL_CACHE", "1")

import ml_dtypes

import concourse.bass as bass
import concourse.tile as tile
from concourse import bacc, mybir
from concourse.bass_utils import run_bass_kernel_spmd

B, S, D, H = 2, 2048, 1024, 16
DK = D // H          # 64
NCORES = 8
HPC = H // (NCORES // B)   # heads per core = 4
DSH = HPC * DK             # 256: per-core shard of the model dim
NKC = S // 128             # 16 key chunks of 128
TRI_W = 384 + 512          # causal strip width

BF = mybir.dt.bfloat16
F32 = mybir.dt.float32
EXP = mybir.ActivationFunctionType.Exp

_NC_CACHE: list = []


def _emit(tc: tile.TileContext, ctx):
    nc = tc.nc

    xT = nc.dram_tensor("xT", [D, S], BF, kind="ExternalInput").ap()
    wqt = nc.dram_tensor("wqt", [D, DSH], BF, kind="ExternalInput").ap()
    wkt = nc.dram_tensor("wkt", [D, DSH], BF, kind="ExternalInput").ap()
    wvt = nc.dram_tensor("wvt", [D, DSH], BF, kind="ExternalInput").ap()
    wot = nc.dram_tensor("wot", [DSH, D], BF, kind="ExternalInput").ap()
    pad0 = nc.dram_tensor("pad0", [128, NKC], F32, kind="ExternalInput").ap()
    padb = nc.dram_tensor("padb", [128, NKC], BF, kind="ExternalInput").ap()
    tri = nc.dram_tensor("tri", [128, TRI_W], BF, kind="ExternalInput").ap()
    yT = nc.dram_tensor("yT", [D, S], BF, kind="ExternalOutput").ap()

    persist = ctx.enter_context(tc.tile_pool(name="persist", bufs=1))
    sc_pool = ctx.enter_context(tc.tile_pool(name="scps", bufs=2, space="PSUM"))
    ct_pool = ctx.enter_context(tc.tile_pool(name="ctps", bufs=2, space="PSUM"))
    pj_pool = ctx.enter_context(tc.tile_pool(name="pjps", bufs=2, space="PSUM"))
    pu_pool = ctx.enter_context(tc.tile_pool(name="pu", bufs=3))
    work = ctx.enter_context(tc.tile_pool(name="work", bufs=4))
    dpool = ctx.enter_context(tc.tile_pool(name="dram", bufs=1, space="DRAM"))

    xs = persist.tile([128, 8, S], BF)
    scratch = persist.tile([128, 256], BF)
    wq_s = persist.tile([128, 8, DSH], BF)
    wk_s = persist.tile([128, 8, DSH], BF)
    wv_s = persist.tile([128, 8, DSH], BF)
    wo_s = persist.tile([128, 2, D], BF)
    pad_s = persist.tile([128, NKC], F32)
    tri_s = persist.tile([128, TRI_W], BF)
    qt2 = persist.tile([128, 2, S], BF)
    kt2 = persist.tile([128, 2, S], BF)
    vp = persist.tile([128, NKC, 65 * HPC], BF)
    ctn = persist.tile([128, 2, S], BF)
    ctu = persist.tile([65, 16, 512], BF)     # unnormalized ctx + L, per (h, qt)
    lpack = persist.tile([16, 8, 64], BF)     # packed L per strip: 2 heads x 8x64
    ldram = dpool.tile([8, 1024], BF)         # recips per strip, head-major

    # ---- input DMAs: what strip (0,0) needs comes first ----
    xr = xT.rearrange("(c p) s -> p c s", p=128)
    wqr = wqt.rearrange("(c p) j -> p c j", p=128)
    wkr = wkt.rearrange("(c p) j -> p c j", p=128)
    wvr = wvt.rearrange("(c p) j -> p c j", p=128)
    wor = wot.rearrange("(c p) o -> p c o", p=128)

    nc.sync.dma_start(out=pad_s, in_=pad0)
    nc.sync.dma_start(out=tri_s, in_=tri)
    # ones columns of vp: broadcast the bf16 pad column into all 4 head slots
    # on the vector engine (a strided 2-byte DMA would hog the ring for ~15us)
    padb_s = persist.tile([128, NKC], BF)
    nc.sync.dma_start(out=padb_s, in_=padb)
    vp_ones = vp.rearrange("p k (h u) -> p k h u", u=65)[:, :, :, 64:65]
    padb_b = bass.AP(
        tensor=padb_s.tensor, offset=padb_s.offset,
        ap=[list(padb_s.ap[0]), list(padb_s.ap[1]), [0, HPC], [0, 1]],
    )
    nc.vector.tensor_copy(out=vp_ones, in_=padb_b)

    # Critical inputs (wk/xs-st0/wq/wv) fan out over 4 queues — scalar and
    # vector help only here, finishing their issues well before the first
    # exp/copy needs those queues. The not-yet-needed xs s-tiles 1-3 issue
    # strictly after, on gpsimd only.
    early = [nc.sync, nc.gpsimd, nc.scalar]
    ei = 0

    def dma_early(out, in_):
        nonlocal ei
        early[ei % len(early)].dma_start(out=out, in_=in_)
        ei += 1

    for c in range(8):
        dma_early(wk_s[:, c, :], wkr[:, c, :])
        dma_early(xs[:, c, 0:512], xr[:, c, 0:512])
        dma_early(wq_s[:, c, :], wqr[:, c, :])
        dma_early(wv_s[:, c, :], wvr[:, c, :])
        ei += 1  # co-prime rotation: alternate which queue gets each type
    for st in range(1, 4):
        for c in range(8):
            nc.gpsimd.dma_start(
                out=xs[:, c, 512 * st : 512 * st + 512],
                in_=xr[:, c, 512 * st : 512 * st + 512],
            )
    for c in range(2):
        nc.gpsimd.dma_start(out=wo_s[:, c, :], in_=wor[:, c, :])

    # ---- emission helpers ----
    def qk_half(hp, wi, st, half, state):
        """Half (4 dc) of one Q/K [128, 512] s-tile projection for pair hp."""
        wsb, dst = ((wq_s, qt2), (wk_s, kt2))[wi]
        if half == 0:
            state["ps"] = pj_pool.tile(
                [128, 512], F32, tag="pj", name=f"qk{hp}{wi}{st}"
            )
        ps = state["ps"]
        for dc in range(4 * half, 4 * half + 4):
            nc.tensor.matmul(
                ps,
                wsb[:, dc, 128 * hp : 128 * hp + 128],
                xs[:, dc, 512 * st : 512 * st + 512],
                start=(dc == 0),
                stop=(dc == 7),
            )
        if half == 1:
            nc.vector.tensor_copy(
                out=dst[:, hp, 512 * st : 512 * st + 512], in_=ps
            )

    def qk_group(hp, wi, st):
        state = {}
        qk_half(hp, wi, st, 0, state)
        qk_half(hp, wi, st, 1, state)

    def v_half(sc, half, state):
        """Half (4 dc) of V key-chunk sc projection + padding mask on finish."""
        if half == 0:
            state["ps"] = pj_pool.tile([128, 512], F32, tag="pj", name=f"v{sc}")
        ps = state["ps"]
        for dc in range(4 * half, 4 * half + 4):
            nc.tensor.matmul(
                ps[:, 0:256],
                xs[:, dc, 128 * sc : 128 * sc + 128],
                wv_s[:, dc, :],
                start=(dc == 0),
                stop=(dc == 7),
            )
        if half == 1:
            dst = vp[:, sc, :].rearrange("p (h u) -> p h u", u=65)[:, :, 0:64]
            src = ps[:, 0:256].rearrange("p (h u) -> p h u", u=64)
            nc.vector.tensor_scalar_mul(dst, src, pad_s[:, sc : sc + 1])

    def v_group(sc):
        state = {}
        v_half(sc, 0, state)
        v_half(sc, 1, state)

    yr = yT.rearrange("(ot p) s -> ot p s", p=128)

    attn_done = [False]

    def out_unit(st, ot, tail):
        # post-attention, the sc psum banks are free: deepen the rotation so
        # MM pairs never wait on the staging copies (keeps HAM warm in the tail)
        if attn_done[0] and ot % 2 == 0:
            ps = sc_pool.tile([128, 1024], F32, tag="sc", name=f"o{st}{ot}")[:, 0:512]
        else:
            ps = pj_pool.tile([128, 512], F32, tag="pj", name=f"o{st}{ot}")
        for c2 in range(2):
            nc.tensor.matmul(
                ps,
                wo_s[:, c2, 128 * ot : 128 * ot + 128],
                ctn[:, c2, 512 * st : 512 * st + 512],
                start=(c2 == 0),
                stop=(c2 == 1),
            )
        ystg = work.tile([128, 512], BF, tag="y", name=f"ys{st}{ot}")
        if tail and ot % 2 == 0:
            nc.scalar.copy(ystg, ps)
        else:
            nc.vector.tensor_copy(out=ystg, in_=ps)
        dq = nc.gpsimd if ot % 2 == 0 else nc.sync
        dq.dma_start(out=yr[ot, :, 512 * st : 512 * st + 512], in_=ystg)

    # ---- filler scheduler ----
    filler: list = []   # (key, cost_ns, emit_fn)
    done_keys = set()

    def emit_next():
        key, cost, fn = filler.pop(0)
        fn()
        done_keys.add(key)
        return cost

    def need(keys):
        keys = [k for k in keys if k not in done_keys]
        forced = False
        while keys:
            emit_next()
            forced = True
            keys = [k for k in keys if k not in done_keys]
        if forced:
            budget[0] = 0.0

    budget = [0.0]

    def run_filler():
        while filler and budget[0] > 0:
            budget[0] -= emit_next()

    # ---- attention chunk stream (software-pipelined across strips) ----
    pending_norm: list = []   # deferred per-strip normalize closures

    def emit_scores(ch):
        hp, qt, kc, w, qs, si = ch["hp"], ch["qt"], ch["kc"], ch["w"], ch["qs"], ch["si"]
        K0 = 128 * kc
        sc = sc_pool.tile([128, 1024], F32, tag="sc", name=f"sc{si}_{kc}")
        nc.tensor.matmul(
            sc[:, 0 : w], kt2[0:64, hp, K0 : K0 + 128],
            qt2[0:64, hp, qs : qs + w],
            start=True, stop=True,
        )
        nc.tensor.matmul(
            sc[:, 512 : 512 + w], kt2[64:128, hp, K0 : K0 + 128],
            qt2[64:128, hp, qs : qs + w],
            start=True, stop=True,
        )
        pu = pu_pool.tile([128, 1024], BF, tag="pu", name=f"pu{si}_{kc}")
        sc2 = sc.rearrange("p (t f) -> p t f", t=2)[:, :, 0:w]
        pu2 = pu.rearrange("p (t f) -> p t f", t=2)[:, :, 0:w]
        nc.scalar.activation(out=pu2, in_=sc2, func=EXP, scale=0.125)
        if ch["band"]:
            tsl = tri_s[:, 384 : 384 + w]
            tri_b = bass.AP(
                tensor=tsl.tensor, offset=tsl.offset,
                ap=[list(tsl.ap[0]), [0, 2], list(tsl.ap[1])],
            )
            nc.vector.tensor_mul(pu2, pu2, tri_b)
        ch["pu"] = pu

    def emit_pv(ch):
        hp, kc, w, co = ch["hp"], ch["kc"], ch["w"], ch["co"]
        pu = ch["pu"]
        he, ho = 2 * hp, 2 * hp + 1
        ct_e, ct_o = ch["ct"]
        nc.tensor.matmul(
            ct_e[:, co : co + w],
            vp[:, kc, 65 * he : 65 * he + 65], pu[:, 0:w],
            start=(kc == 0), stop=ch["last"],
        )
        nc.tensor.matmul(
            ct_o[:, co : co + w],
            vp[:, kc, 65 * ho : 65 * ho + 65], pu[:, 512 : 512 + w],
            start=(kc == 0), stop=ch["last"],
        )

    def emit_post(ch):
        """After a strip's last PV: stage ctx+L, pack L, recip, bounce."""
        hp, qt, si = ch["hp"], ch["qt"], ch["si"]
        ct_e, ct_o = ch["ct"]
        for idx, cta in ((0, ct_e), (1, ct_o)):
            hq = (2 * hp + idx) * 4 + qt
            nc.vector.tensor_copy(out=ctu[:, hq, :], in_=cta)
            nc.sync.dma_start(
                out=lpack[8 * idx : 8 * idx + 8, si, :], in_=ctu[64:65, hq, :]
            )
        lp = lpack[:, si, :]
        nc.vector.tensor_scalar_max(lp, lp, 1e-30)
        with nc.allow_low_precision(reason="1/L in bf16: 0.4% rel, budget 2e-2"):
            nc.vector.reciprocal(lp, lp)
        nc.sync.dma_start(out=ldram[si : si + 1, :], in_=lp)

        Q0 = 512 * qt

        def norm_tail(hp=hp, qt=qt, si=si, Q0=Q0):
            for idx in (0, 1):
                hq = (2 * hp + idx) * 4 + qt
                rlb = work.tile([64, 512], BF, tag="rlb", name=f"rlb{si}{idx}")
                row = ldram[si : si + 1, 512 * idx : 512 * idx + 512]
                bsrc = bass.AP(
                    tensor=row.tensor, offset=row.offset,
                    ap=[[0, 64]] + list(row.ap[1:]),
                )
                nc.sync.dma_start(out=rlb, in_=bsrc)
                if idx == 0:
                    nc.vector.tensor_mul(
                        ctn[0:64, hp, Q0 : Q0 + 512], ctu[0:64, hq, :], rlb
                    )
                else:
                    stg = work.tile([64, 512], BF, tag="stg", name=f"stg{si}")
                    nc.vector.tensor_mul(stg, ctu[0:64, hq, :], rlb)
                    nc.sync.dma_start(
                        out=ctn[64:128, hp, Q0 : Q0 + 512], in_=stg
                    )

        pending_norm.append((hp, qt, norm_tail))

    # ---- build filler queue ----
    def add_qk(hp, wi, st):
        state = {}
        filler.append(
            (("qk2", hp, wi, st), 915.0,
             lambda hp=hp, wi=wi, st=st, state=state: qk_half(hp, wi, st, 0, state))
        )
        filler.append(
            (("qk", hp, wi, st), 915.0,
             lambda hp=hp, wi=wi, st=st, state=state: qk_half(hp, wi, st, 1, state))
        )

    def add_v(sc):
        state = {}
        filler.append(
            (("v2", sc), 750.0, lambda sc=sc, state=state: v_half(sc, 0, state))
        )
        filler.append(
            (("v", sc), 750.0, lambda sc=sc, state=state: v_half(sc, 1, state))
        )

    def add_out(st, tail=False):
        # out units go to the FRONT: they're ready immediately and small, so
        # they soak up bubbles instead of piling into a cold post-attention
        # tail; displaced qk/v units get force-emitted at their strip boundary
        for ot in reversed(range(8)):
            filler.insert(
                0,
                (("out", st, ot), 480.0,
                 lambda st=st, ot=ot, tail=tail: out_unit(st, ot, tail)),
            )

    add_qk(0, 1, 1)
    add_qk(0, 0, 1)
    for sc in range(4, 8):
        add_v(sc)
    add_qk(1, 1, 1)
    add_qk(1, 0, 1)
    add_qk(0, 1, 2)
    add_qk(0, 0, 2)
    for sc in range(8, 12):
        add_v(sc)
    add_qk(1, 1, 2)
    add_qk(1, 0, 2)
    add_qk(0, 1, 3)
    add_qk(0, 0, 3)
    for sc in range(12, 16):
        add_v(sc)
    add_qk(1, 1, 3)
    add_qk(1, 0, 3)

    # ---- main schedule ----
    # HAM warmup: ~3.4us of dummy matmuls on scratch while the input DMAs
    # land, so the real projections start at 2.4 GHz instead of 1.2
    nc.vector.memset(scratch, 0.0)
    du = pj_pool.tile([128, 512], F32, tag="pj", name="du")
    for _ in range(16):
        nc.tensor.matmul(
            du[:, 0:256], scratch[:, 0:128], scratch, start=True, stop=True
        )

    # pre-phase: both pairs' st0 projections + V chunks 0-3 (the PE is
    # DMA-gated here anyway, so this costs no extra wall time and removes
    # the forced block at the (1,0) strip boundary)
    qk_group(0, 1, 0)
    qk_group(0, 0, 0)
    qk_group(1, 1, 0)
    qk_group(1, 0, 0)
    for sc in range(4):
        v_group(sc)
    done_keys.update(
        {("qk", 0, 1, 0), ("qk", 0, 0, 0), ("qk", 1, 1, 0), ("qk", 1, 0, 0),
         ("v", 0), ("v", 1), ("v", 2), ("v", 3)}
    )

    def flush_norm():
        nhp, nqt, fn = pending_norm.pop(0)
        fn()
        if nhp == 1:
            add_out(nqt, tail=(nqt == 3))

    # flat chunk list over the interleaved strip order
    chunks = []
    for qt in range(4):
        for hp in range(2):
            si = 2 * qt + hp
            Q0 = 512 * qt
            nkc = 4 * qt + 4
            for kc in range(nkc):
                K0 = 128 * kc
                band = K0 >= Q0
                qs = K0 if band else Q0
                chunks.append({
                    "hp": hp, "qt": qt, "kc": kc, "si": si, "band": band,
                    "qs": qs, "w": Q0 + 512 - qs, "co": qs - Q0,
                    "first": kc == 0, "last": kc == nkc - 1,
                })

    prev = None
    for ch in chunks:
        if ch["first"]:
            need(
                [("qk", ch["hp"], 1, st) for st in range(ch["qt"] + 1)]
                + [("qk", ch["hp"], 0, ch["qt"])]
                + [("v", sc) for sc in range(4 * ch["qt"] + 4)]
            )
            si = ch["si"]
            ct = (
                ct_pool.tile([65, 512], F32, tag="ct", name=f"cte{si}"),
                ct_pool.tile([65, 512], F32, tag="ct", name=f"cto{si}"),
            )
        ch["ct"] = ct
        emit_scores(ch)
        if prev is not None:
            budget[0] = min(
                budget[0]
                + (2 * prev["w"] + 352) / 1.2
                - (3 * prev["w"] + 260) / 2.4,
                3000.0,
            )
            run_filler()
            emit_pv(prev)
            if prev["last"]:
                emit_post(prev)
                # flush the strip-before-last's normalize (its DRAM bounce
                # has had a full strip of time)
                if len(pending_norm) >= 2:
                    flush_norm()
        prev = ch
    emit_pv(prev)
    emit_post(prev)
    attn_done[0] = True
    while pending_norm:
        flush_norm()
    # drain remaining filler (out_proj of late qts)
    while filler:
        emit_next()


def build_nc():
    nc = bacc.Bacc(
        "TRN2",
        target_bir_lowering=False,
        debug=False,
        enable_asserts=False,
        num_devices=NCORES,
    )
    from contextlib import ExitStack

    with tile.TileContext(nc) as tc:
        with ExitStack() as ctx:
            _emit(tc, ctx)
    nc.compile()
    return nc


def _get_nc():
    if not _NC_CACHE:
        _NC_CACHE.append(build_nc())
    return _NC_CACHE[0]


def make_tri() -> np.ndarray:
    p = np.arange(128)[:, None]
    v = np.arange(TRI_W)[None, :]
    return (p <= v - 384).astype(np.float32).astype(ml_dtypes.bfloat16)


def make_in_maps(x, mask, WQ, WK, WV, WO):
    bf = ml_dtypes.bfloat16
    tri = make_tri()
    in_maps = []
    for c in range(NCORES):
        b, g = c // (NCORES // B), c % (NCORES // B)
        sl = slice(DSH * g, DSH * g + DSH)
        pad = np.ascontiguousarray(
            (mask[b] == 0).astype(np.float32).reshape(NKC, 128).T
        )
        in_maps.append(
            {
                "xT": np.ascontiguousarray(x[b].T).astype(bf),
                "wqt": np.ascontiguousarray(WQ[sl, :].T).astype(bf),
                "wkt": np.ascontiguousarray(WK[sl, :].T).astype(bf),
                "wvt": np.ascontiguousarray(WV[sl, :].T).astype(bf),
                "wot": np.ascontiguousarray(WO[:, sl].T).astype(bf),
                "pad0": pad,
                "padb": pad.astype(bf),
                "tri": tri,
            }
        )
    return in_maps


def assemble(results, x, mask, WV, WO, bO) -> np.ndarray:
    y = np.zeros((B, S, D), np.float32)
    for c in range(NCORES):
        y[c // (NCORES // B)] += results[c]["yT"].T
    y += bO[None, None, :]
    # Rows i < first-unmasked-index are fully masked in the reference; its
    # softmax then degenerates to uniform attention over all positions.
    for b in range(B):
        nz = np.nonzero(mask[b] == 0)[0]
        t = int(nz[0]) if nz.size else S
        if t > 0:
            vbar = x[b].mean(axis=0) @ WV.T
            yfix = vbar @ WO.T + bO
            y[b, :t, :] = yfix
    return y


def kernel(x, mask, WQ, WK, WV, WO, bO) -> np.ndarray:
    x = np.asarray(x, np.float32)
    mask = np.asarray(mask, np.int32)
    WQ = np.asarray(WQ, np.float32)
    WK = np.asarray(WK, np.float32)
    WV = np.asarray(WV, np.float32)
    WO = np.asarray(WO, np.float32)
    bO = np.asarray(bO, np.float32)

    nc = _get_nc()
    in_maps = make_in_maps(x, mask, WQ, WK, WV, WO)
    res = run_bass_kernel_spmd(nc, in_maps, list(range(NCORES)))
    return assemble(res.results, x, mask, WV, WO, bO)


# revision 28
# speedup vs baseline: 1.0313x; 1.0313x over previous
"""Multi-head attention (B=2, S=2048, D=1024, H=16) on 8 NeuronCores.

Sharding: core c -> (batch b = c // 4, head-group g = c % 4, 4 heads each).
Each core computes its 4 heads' attention for its batch plus the partial
output projection (ctx_shard @ WO_shard.T).T; the host sums the 4 partials
per batch, adds the bias, and patches fully-masked query rows (where the
reference's softmax degenerates to uniform attention).

v2 schedule: the exp stream on the scalar engine is the pacer (~80us of
activation work), so the kernel is organized as one continuous attention
stream that starts as early as possible and hides everything else inside it:
  - Strips run interleaved (hp,qt) = (0,0),(1,0),(0,1),(1,1),... so each
    qt's normalization + output projection pipeline into later strips.
  - Only Q/K s-tile 0 and V chunks 0-3 are projected up front; all other
    projection matmuls are emitted as "filler" between attention chunks,
    sized to the per-chunk exp-minus-PE bubble, keeping the PE dense (HAM
    stays warm) without starving the scores->exp chain.
  - Scalar engine does exp only during the stream; V-projection copy+mask
    is one vector op per chunk; the V ones-columns are filled by a single
    broadcast DMA of the bf16 padding mask.
  - Softmax normalization is deferred: V carries a ones column so P@V also
    accumulates row sums L[q]; per strip, L rows are DMA-packed into a
    [16,64] tile (fast reciprocal), bounced through DRAM for a partition
    broadcast, and divided into ctx while later strips run.
"""

import os
import sys

import numpy as np

sys.path.insert(0, "/opt/trn_rl_repo")
os.environ.setdefault("MYCRO_LOCAL_CACHE", "1")

import ml_dtypes

import concourse.bass as bass
import concourse.tile as tile
from concourse import bacc, mybir
from concourse.bass_utils import run_bass_kernel_spmd

B, S, D, H = 2, 2048, 1024, 16
DK = D // H          # 64
NCORES = 8
HPC = H // (NCORES // B)   # heads per core = 4
DSH = HPC * DK             # 256: per-core shard of the model dim
NKC = S // 128             # 16 key chunks of 128
TRI_W = 384 + 512          # causal strip width

BF = mybir.dt.bfloat16
F32 = mybir.dt.float32
EXP = mybir.ActivationFunctionType.Exp

_NC_CACHE: list = []


def _emit(tc: tile.TileContext, ctx):
    nc = tc.nc

    xT = nc.dram_tensor("xT", [D, S], BF, kind="ExternalInput").ap()
    wqt = nc.dram_tensor("wqt", [D, DSH], BF, kind="ExternalInput").ap()
    wkt = nc.dram_tensor("wkt", [D, DSH], BF, kind="ExternalInput").ap()
    wvt = nc.dram_tensor("wvt", [D, DSH], BF, kind="ExternalInput").ap()
    wot = nc.dram_tensor("wot", [DSH, D], BF, kind="ExternalInput").ap()
    pad0 = nc.dram_tensor("pad0", [128, NKC], F32, kind="ExternalInput").ap()
    padb = nc.dram_tensor("padb", [128, NKC], BF, kind="ExternalInput").ap()
    tri = nc.dram_tensor("tri", [128, TRI_W], BF, kind="ExternalInput").ap()
    yT = nc.dram_tensor("yT", [D, S], BF, kind="ExternalOutput").ap()

    persist = ctx.enter_context(tc.tile_pool(name="persist", bufs=1))
    sc_pool = ctx.enter_context(tc.tile_pool(name="scps", bufs=2, space="PSUM"))
    ct_pool = ctx.enter_context(tc.tile_pool(name="ctps", bufs=2, space="PSUM"))
    pj_pool = ctx.enter_context(tc.tile_pool(name="pjps", bufs=2, space="PSUM"))
    pu_pool = ctx.enter_context(tc.tile_pool(name="pu", bufs=3))
    work = ctx.enter_context(tc.tile_pool(name="work", bufs=4))
    dpool = ctx.enter_context(tc.tile_pool(name="dram", bufs=1, space="DRAM"))

    xs = persist.tile([128, 8, S], BF)
    scratch = persist.tile([128, 256], BF)
    wq_s = persist.tile([128, 8, DSH], BF)
    wk_s = persist.tile([128, 8, DSH], BF)
    wv_s = persist.tile([128, 8, DSH], BF)
    wo_s = persist.tile([128, 2, D], BF)
    pad_s = persist.tile([128, NKC], F32)
    tri_s = persist.tile([128, TRI_W], BF)
    qt2 = persist.tile([128, 2, S], BF)
    kt2 = persist.tile([128, 2, S], BF)
    vp = persist.tile([128, NKC, 65 * HPC], BF)
    ctn = persist.tile([128, 2, S], BF)
    ctu = persist.tile([65, 16, 512], BF)     # unnormalized ctx + L, per (h, qt)
    lpack = persist.tile([16, 8, 64], BF)     # packed L per strip: 2 heads x 8x64
    ldram = dpool.tile([8, 1024], BF)         # recips per strip, head-major

    # ---- input DMAs: what strip (0,0) needs comes first ----
    xr = xT.rearrange("(c p) s -> p c s", p=128)
    wqr = wqt.rearrange("(c p) j -> p c j", p=128)
    wkr = wkt.rearrange("(c p) j -> p c j", p=128)
    wvr = wvt.rearrange("(c p) j -> p c j", p=128)
    wor = wot.rearrange("(c p) o -> p c o", p=128)

    nc.sync.dma_start(out=pad_s, in_=pad0)
    nc.sync.dma_start(out=tri_s, in_=tri)
    # ones columns of vp: broadcast the bf16 pad column into all 4 head slots
    # on the vector engine (a strided 2-byte DMA would hog the ring for ~15us)
    padb_s = persist.tile([128, NKC], BF)
    nc.sync.dma_start(out=padb_s, in_=padb)
    vp_ones = vp.rearrange("p k (h u) -> p k h u", u=65)[:, :, :, 64:65]
    padb_b = bass.AP(
        tensor=padb_s.tensor, offset=padb_s.offset,
        ap=[list(padb_s.ap[0]), list(padb_s.ap[1]), [0, HPC], [0, 1]],
    )
    nc.vector.tensor_copy(out=vp_ones, in_=padb_b)

    # Input DMAs on sync/gpsimd/scalar (scalar's issues all land before the
    # first exp needs its queue).
    early = [nc.sync, nc.gpsimd, nc.scalar]
    ei = 0

    def dma_early(out, in_):
        nonlocal ei
        early[ei % len(early)].dma_start(out=out, in_=in_)
        ei += 1

    # strict need-order: K proj needs wk+xs-st0, then Q needs wq, then V
    # needs wv; the rest of xs round-robins over the same queues so no queue
    # races ahead and steals HBM bandwidth from the critical set
    for c in range(8):
        dma_early(wk_s[:, c, :], wkr[:, c, :])
        dma_early(xs[:, c, 0:512], xr[:, c, 0:512])
        ei += 1  # co-prime rotation: alternate which queue gets each type
    for c in range(8):
        dma_early(wq_s[:, c, :], wqr[:, c, :])
    for c in range(8):
        dma_early(wv_s[:, c, :], wvr[:, c, :])
    for st in range(1, 4):
        for c in range(8):
            dma_early(
                xs[:, c, 512 * st : 512 * st + 512],
                xr[:, c, 512 * st : 512 * st + 512],
            )
    for c in range(2):
        dma_early(wo_s[:, c, :], wor[:, c, :])

    # ---- emission helpers ----
    def qk_half(hp, wi, st, half, state):
        """Half (4 dc) of one Q/K [128, 512] s-tile projection for pair hp."""
        wsb, dst = ((wq_s, qt2), (wk_s, kt2))[wi]
        if half == 0:
            state["ps"] = pj_pool.tile(
                [128, 512], F32, tag="pj", name=f"qk{hp}{wi}{st}"
            )
        ps = state["ps"]
        for dc in range(4 * half, 4 * half + 4):
            nc.tensor.matmul(
                ps,
                wsb[:, dc, 128 * hp : 128 * hp + 128],
                xs[:, dc, 512 * st : 512 * st + 512],
                start=(dc == 0),
                stop=(dc == 7),
            )
        if half == 1:
            nc.vector.tensor_copy(
                out=dst[:, hp, 512 * st : 512 * st + 512], in_=ps
            )

    def qk_group(hp, wi, st):
        state = {}
        qk_half(hp, wi, st, 0, state)
        qk_half(hp, wi, st, 1, state)

    def v_half(sc, half, state):
        """Half (4 dc) of V key-chunk sc projection + padding mask on finish."""
        if half == 0:
            state["ps"] = pj_pool.tile([128, 512], F32, tag="pj", name=f"v{sc}")
        ps = state["ps"]
        for dc in range(4 * half, 4 * half + 4):
            nc.tensor.matmul(
                ps[:, 0:256],
                xs[:, dc, 128 * sc : 128 * sc + 128],
                wv_s[:, dc, :],
                start=(dc == 0),
                stop=(dc == 7),
            )
        if half == 1:
            dst = vp[:, sc, :].rearrange("p (h u) -> p h u", u=65)[:, :, 0:64]
            src = ps[:, 0:256].rearrange("p (h u) -> p h u", u=64)
            nc.vector.tensor_scalar_mul(dst, src, pad_s[:, sc : sc + 1])

    def v_group(sc):
        state = {}
        v_half(sc, 0, state)
        v_half(sc, 1, state)

    yr = yT.rearrange("(ot p) s -> ot p s", p=128)

    attn_done = [False]

    def out_unit(st, ot, tail):
        # post-attention, the sc psum banks are free: deepen the rotation so
        # MM pairs never wait on the staging copies (keeps HAM warm in the tail)
        if attn_done[0] and ot % 2 == 0:
            ps = sc_pool.tile([128, 1024], F32, tag="sc", name=f"o{st}{ot}")[:, 0:512]
        else:
            ps = pj_pool.tile([128, 512], F32, tag="pj", name=f"o{st}{ot}")
        for c2 in range(2):
            nc.tensor.matmul(
                ps,
                wo_s[:, c2, 128 * ot : 128 * ot + 128],
                ctn[:, c2, 512 * st : 512 * st + 512],
                start=(c2 == 0),
                stop=(c2 == 1),
            )
        ystg = work.tile([128, 512], BF, tag="y", name=f"ys{st}{ot}")
        if tail and ot % 2 == 0:
            nc.scalar.copy(ystg, ps)
        else:
            nc.vector.tensor_copy(out=ystg, in_=ps)
        dq = nc.gpsimd if ot % 2 == 0 else nc.sync
        dq.dma_start(out=yr[ot, :, 512 * st : 512 * st + 512], in_=ystg)

    # ---- filler scheduler ----
    filler: list = []   # (key, cost_ns, emit_fn)
    done_keys = set()

    def emit_next():
        key, cost, fn = filler.pop(0)
        fn()
        done_keys.add(key)
        return cost

    def need(keys):
        keys = [k for k in keys if k not in done_keys]
        forced = False
        while keys:
            emit_next()
            forced = True
            keys = [k for k in keys if k not in done_keys]
        if forced:
            budget[0] = 0.0

    budget = [0.0]

    def run_filler():
        while filler and budget[0] > 0:
            budget[0] -= emit_next()

    # ---- attention chunk stream (software-pipelined across strips) ----
    pending_norm: list = []   # deferred per-strip normalize closures

    def emit_scores(ch):
        hp, qt, kc, w, qs, si = ch["hp"], ch["qt"], ch["kc"], ch["w"], ch["qs"], ch["si"]
        K0 = 128 * kc
        sc = sc_pool.tile([128, 1024], F32, tag="sc", name=f"sc{si}_{kc}")
        nc.tensor.matmul(
            sc[:, 0 : w], kt2[0:64, hp, K0 : K0 + 128],
            qt2[0:64, hp, qs : qs + w],
            start=True, stop=True,
        )
        nc.tensor.matmul(
            sc[:, 512 : 512 + w], kt2[64:128, hp, K0 : K0 + 128],
            qt2[64:128, hp, qs : qs + w],
            start=True, stop=True,
        )
        pu = pu_pool.tile([128, 1024], BF, tag="pu", name=f"pu{si}_{kc}")
        sc2 = sc.rearrange("p (t f) -> p t f", t=2)[:, :, 0:w]
        pu2 = pu.rearrange("p (t f) -> p t f", t=2)[:, :, 0:w]
        nc.scalar.activation(out=pu2, in_=sc2, func=EXP, scale=0.125)
        if ch["band"]:
            tsl = tri_s[:, 384 : 384 + w]
            tri_b = bass.AP(
                tensor=tsl.tensor, offset=tsl.offset,
                ap=[list(tsl.ap[0]), [0, 2], list(tsl.ap[1])],
            )
            nc.vector.tensor_mul(pu2, pu2, tri_b)
        ch["pu"] = pu

    def emit_pv(ch):
        hp, kc, w, co = ch["hp"], ch["kc"], ch["w"], ch["co"]
        pu = ch["pu"]
        he, ho = 2 * hp, 2 * hp + 1
        ct_e, ct_o = ch["ct"]
        nc.tensor.matmul(
            ct_e[:, co : co + w],
            vp[:, kc, 65 * he : 65 * he + 65], pu[:, 0:w],
            start=(kc == 0), stop=ch["last"],
        )
        nc.tensor.matmul(
            ct_o[:, co : co + w],
            vp[:, kc, 65 * ho : 65 * ho + 65], pu[:, 512 : 512 + w],
            start=(kc == 0), stop=ch["last"],
        )

    def emit_post(ch):
        """After a strip's last PV: stage ctx+L, pack L, recip, bounce."""
        hp, qt, si = ch["hp"], ch["qt"], ch["si"]
        ct_e, ct_o = ch["ct"]
        for idx, cta in ((0, ct_e), (1, ct_o)):
            hq = (2 * hp + idx) * 4 + qt
            nc.vector.tensor_copy(out=ctu[:, hq, :], in_=cta)
            nc.sync.dma_start(
                out=lpack[8 * idx : 8 * idx + 8, si, :], in_=ctu[64:65, hq, :]
            )
        lp = lpack[:, si, :]
        nc.vector.tensor_scalar_max(lp, lp, 1e-30)
        with nc.allow_low_precision(reason="1/L in bf16: 0.4% rel, budget 2e-2"):
            nc.vector.reciprocal(lp, lp)
        nc.sync.dma_start(out=ldram[si : si + 1, :], in_=lp)

        Q0 = 512 * qt

        def norm_tail(hp=hp, qt=qt, si=si, Q0=Q0):
            for idx in (0, 1):
                hq = (2 * hp + idx) * 4 + qt
                rlb = work.tile([64, 512], BF, tag="rlb", name=f"rlb{si}{idx}")
                row = ldram[si : si + 1, 512 * idx : 512 * idx + 512]
                bsrc = bass.AP(
                    tensor=row.tensor, offset=row.offset,
                    ap=[[0, 64]] + list(row.ap[1:]),
                )
                nc.sync.dma_start(out=rlb, in_=bsrc)
                if idx == 0:
                    nc.vector.tensor_mul(
                        ctn[0:64, hp, Q0 : Q0 + 512], ctu[0:64, hq, :], rlb
                    )
                else:
                    stg = work.tile([64, 512], BF, tag="stg", name=f"stg{si}")
                    nc.vector.tensor_mul(stg, ctu[0:64, hq, :], rlb)
                    nc.sync.dma_start(
                        out=ctn[64:128, hp, Q0 : Q0 + 512], in_=stg
                    )

        pending_norm.append((hp, qt, norm_tail))

    # ---- build filler queue ----
    def add_qk(hp, wi, st):
        state = {}
        filler.append(
            (("qk2", hp, wi, st), 915.0,
             lambda hp=hp, wi=wi, st=st, state=state: qk_half(hp, wi, st, 0, state))
        )
        filler.append(
            (("qk", hp, wi, st), 915.0,
             lambda hp=hp, wi=wi, st=st, state=state: qk_half(hp, wi, st, 1, state))
        )

    def add_v(sc):
        state = {}
        filler.append(
            (("v2", sc), 750.0, lambda sc=sc, state=state: v_half(sc, 0, state))
        )
        filler.append(
            (("v", sc), 750.0, lambda sc=sc, state=state: v_half(sc, 1, state))
        )

    def add_out(st, tail=False):
        for ot in range(8):
            filler.append(
                (("out", st, ot), 480.0,
                 lambda st=st, ot=ot, tail=tail: out_unit(st, ot, tail))
            )

    add_qk(0, 1, 1)
    add_qk(0, 0, 1)
    for sc in range(4, 8):
        add_v(sc)
    add_qk(1, 1, 1)
    add_qk(1, 0, 1)
    add_qk(0, 1, 2)
    add_qk(0, 0, 2)
    for sc in range(8, 12):
        add_v(sc)
    add_qk(1, 1, 2)
    add_qk(1, 0, 2)
    add_qk(0, 1, 3)
    add_qk(0, 0, 3)
    for sc in range(12, 16):
        add_v(sc)
    add_qk(1, 1, 3)
    add_qk(1, 0, 3)

    # ---- main schedule ----
    # HAM warmup: ~3.4us of dummy matmuls on scratch while the input DMAs
    # land, so the real projections start at 2.4 GHz instead of 1.2
    nc.vector.memset(scratch, 0.0)
    du = pj_pool.tile([128, 512], F32, tag="pj", name="du")
    for _ in range(16):
        nc.tensor.matmul(
            du[:, 0:256], scratch[:, 0:128], scratch, start=True, stop=True
        )

    # pre-phase: both pairs' st0 projections + V chunks 0-3 (the PE is
    # DMA-gated here anyway, so this costs no extra wall time and removes
    # the forced block at the (1,0) strip boundary)
    qk_group(0, 1, 0)
    qk_group(0, 0, 0)
    qk_group(1, 1, 0)
    qk_group(1, 0, 0)
    for sc in range(4):
        v_group(sc)
    done_keys.update(
        {("qk", 0, 1, 0), ("qk", 0, 0, 0), ("qk", 1, 1, 0), ("qk", 1, 0, 0),
         ("v", 0), ("v", 1), ("v", 2), ("v", 3)}
    )

    def flush_norm():
        nhp, nqt, fn = pending_norm.pop(0)
        fn()
        if nhp == 1:
            add_out(nqt, tail=(nqt == 3))

    # flat chunk list over the interleaved strip order
    chunks = []
    for qt in range(4):
        for hp in range(2):
            si = 2 * qt + hp
            Q0 = 512 * qt
            nkc = 4 * qt + 4
            for kc in range(nkc):
                K0 = 128 * kc
                band = K0 >= Q0
                qs = K0 if band else Q0
                chunks.append({
                    "hp": hp, "qt": qt, "kc": kc, "si": si, "band": band,
                    "qs": qs, "w": Q0 + 512 - qs, "co": qs - Q0,
                    "first": kc == 0, "last": kc == nkc - 1,
                })

    prev = None
    for ch in chunks:
        if ch["first"]:
            need(
                [("qk", ch["hp"], 1, st) for st in range(ch["qt"] + 1)]
                + [("qk", ch["hp"], 0, ch["qt"])]
                + [("v", sc) for sc in range(4 * ch["qt"] + 4)]
            )
            si = ch["si"]
            ct = (
                ct_pool.tile([65, 512], F32, tag="ct", name=f"cte{si}"),
                ct_pool.tile([65, 512], F32, tag="ct", name=f"cto{si}"),
            )
        ch["ct"] = ct
        emit_scores(ch)
        if prev is not None:
            budget[0] = min(
                budget[0]
                + (2 * prev["w"] + 352) / 1.2
                - (3 * prev["w"]) / 2.4,
                3000.0,
            )
            run_filler()
            emit_pv(prev)
            if prev["last"]:
                emit_post(prev)
                # flush the strip-before-last's normalize (its DRAM bounce
                # has had a full strip of time)
                if len(pending_norm) >= 2:
                    flush_norm()
        prev = ch
    emit_pv(prev)
    emit_post(prev)
    attn_done[0] = True
    while pending_norm:
        flush_norm()
    # drain remaining filler (out_proj of late qts)
    while filler:
        emit_next()


def build_nc():
    nc = bacc.Bacc(
        "TRN2",
        target_bir_lowering=False,
        debug=False,
        enable_asserts=False,
        num_devices=NCORES,
    )
    from contextlib import ExitStack

    with tile.TileContext(nc) as tc:
        with ExitStack() as ctx:
            _emit(tc, ctx)
    nc.compile()
    return nc


def _get_nc():
    if not _NC_CACHE:
        _NC_CACHE.append(build_nc())
    return _NC_CACHE[0]


def make_tri() -> np.ndarray:
    p = np.arange(128)[:, None]
    v = np.arange(TRI_W)[None, :]
    return (p <= v - 384).astype(np.float32).astype(ml_dtypes.bfloat16)


def make_in_maps(x, mask, WQ, WK, WV, WO):
    bf = ml_dtypes.bfloat16
    tri = make_tri()
    in_maps = []
    for c in range(NCORES):
        b, g = c // (NCORES // B), c % (NCORES // B)
        sl = slice(DSH * g, DSH * g + DSH)
        pad = np.ascontiguousarray(
            (mask[b] == 0).astype(np.float32).reshape(NKC, 128).T
        )
        in_maps.append(
            {
                "xT": np.ascontiguousarray(x[b].T).astype(bf),
                "wqt": np.ascontiguousarray(WQ[sl, :].T).astype(bf),
                "wkt": np.ascontiguousarray(WK[sl, :].T).astype(bf),
                "wvt": np.ascontiguousarray(WV[sl, :].T).astype(bf),
                "wot": np.ascontiguousarray(WO[:, sl].T).astype(bf),
                "pad0": pad,
                "padb": pad.astype(bf),
                "tri": tri,
            }
        )
    return in_maps


def assemble(results, x, mask, WV, WO, bO) -> np.ndarray:
    y = np.zeros((B, S, D), np.float32)
    for c in range(NCORES):
        y[c // (NCORES // B)] += results[c]["yT"].T
    y += bO[None, None, :]
    # Rows i < first-unmasked-index are fully masked in the reference; its
    # softmax then degenerates to uniform attention over all positions.
    for b in range(B):
        nz = np.nonzero(mask[b] == 0)[0]
        t = int(nz[0]) if nz.size else S
        if t > 0:
            vbar = x[b].mean(axis=0) @ WV.T
            yfix = vbar @ WO.T + bO
            y[b, :t, :] = yfix
    return y


def kernel(x, mask, WQ, WK, WV, WO, bO) -> np.ndarray:
    x = np.asarray(x, np.float32)
    mask = np.asarray(mask, np.int32)
    WQ = np.asarray(WQ, np.float32)
    WK = np.asarray(WK, np.float32)
    WV = np.asarray(WV, np.float32)
    WO = np.asarray(WO, np.float32)
    bO = np.asarray(bO, np.float32)

    nc = _get_nc()
    in_maps = make_in_maps(x, mask, WQ, WK, WV, WO)
    res = run_bass_kernel_spmd(nc, in_maps, list(range(NCORES)))
    return assemble(res.results, x, mask, WV, WO, bO)


# revision 33
# speedup vs baseline: 1.0397x; 1.0082x over previous
"""Multi-head attention (B=2, S=2048, D=1024, H=16) on 8 NeuronCores.

Sharding: core c -> (batch b = c // 4, head-group g = c % 4, 4 heads each).
Each core computes its 4 heads' attention for its batch plus the partial
output projection (ctx_shard @ WO_shard.T).T; the host sums the 4 partials
per batch, adds the bias, and patches fully-masked query rows (where the
reference's softmax degenerates to uniform attention).

v2 schedule: the exp stream on the scalar engine is the pacer (~80us of
activation work), so the kernel is organized as one continuous attention
stream that starts as early as possible and hides everything else inside it:
  - Strips run interleaved (hp,qt) = (0,0),(1,0),(0,1),(1,1),... so each
    qt's normalization + output projection pipeline into later strips.
  - Only Q/K s-tile 0 and V chunks 0-3 are projected up front; all other
    projection matmuls are emitted as "filler" between attention chunks,
    sized to the per-chunk exp-minus-PE bubble, keeping the PE dense (HAM
    stays warm) without starving the scores->exp chain.
  - Scalar engine does exp only during the stream; V-projection copy+mask
    is one vector op per chunk; the V ones-columns are filled by a single
    broadcast DMA of the bf16 padding mask.
  - Softmax normalization is deferred: V carries a ones column so P@V also
    accumulates row sums L[q]; per strip, L rows are DMA-packed into a
    [16,64] tile (fast reciprocal), bounced through DRAM for a partition
    broadcast, and divided into ctx while later strips run.
"""

import os
import sys

import numpy as np

sys.path.insert(0, "/opt/trn_rl_repo")
os.environ.setdefault("MYCRO_LOCAL_CACHE", "1")

import ml_dtypes

import concourse.bass as bass
import concourse.tile as tile
from concourse import bacc, mybir
from concourse.bass_utils import run_bass_kernel_spmd

B, S, D, H = 2, 2048, 1024, 16
DK = D // H          # 64
NCORES = 8
HPC = H // (NCORES // B)   # heads per core = 4
DSH = HPC * DK             # 256: per-core shard of the model dim
NKC = S // 128             # 16 key chunks of 128
TRI_W = 384 + 512          # causal strip width

BF = mybir.dt.bfloat16
F32 = mybir.dt.float32
EXP = mybir.ActivationFunctionType.Exp

_NC_CACHE: list = []


def _emit(tc: tile.TileContext, ctx):
    nc = tc.nc

    xT = nc.dram_tensor("xT", [D, S], BF, kind="ExternalInput").ap()
    wqt = nc.dram_tensor("wqt", [D, DSH], BF, kind="ExternalInput").ap()
    wkt = nc.dram_tensor("wkt", [D, DSH], BF, kind="ExternalInput").ap()
    wvt = nc.dram_tensor("wvt", [D, DSH], BF, kind="ExternalInput").ap()
    wot = nc.dram_tensor("wot", [DSH, D], BF, kind="ExternalInput").ap()
    pad0 = nc.dram_tensor("pad0", [128, NKC], F32, kind="ExternalInput").ap()
    padb = nc.dram_tensor("padb", [128, NKC], BF, kind="ExternalInput").ap()
    tri = nc.dram_tensor("tri", [128, TRI_W], BF, kind="ExternalInput").ap()
    yT = nc.dram_tensor("yT", [D, S], BF, kind="ExternalOutput").ap()

    persist = ctx.enter_context(tc.tile_pool(name="persist", bufs=1))
    sc_pool = ctx.enter_context(tc.tile_pool(name="scps", bufs=2, space="PSUM"))
    ct_pool = ctx.enter_context(tc.tile_pool(name="ctps", bufs=2, space="PSUM"))
    pj_pool = ctx.enter_context(tc.tile_pool(name="pjps", bufs=2, space="PSUM"))
    pu_pool = ctx.enter_context(tc.tile_pool(name="pu", bufs=3))
    work = ctx.enter_context(tc.tile_pool(name="work", bufs=4))
    dpool = ctx.enter_context(tc.tile_pool(name="dram", bufs=1, space="DRAM"))

    xs = persist.tile([128, 8, S], BF)
    scratch = persist.tile([128, 256], BF)
    wq_s = persist.tile([128, 8, DSH], BF)
    wk_s = persist.tile([128, 8, DSH], BF)
    wv_s = persist.tile([128, 8, DSH], BF)
    wo_s = persist.tile([128, 2, D], BF)
    pad_s = persist.tile([128, NKC], F32)
    tri_s = persist.tile([128, TRI_W], BF)
    qt2 = persist.tile([128, 2, S], BF)
    kt2 = persist.tile([128, 2, S], BF)
    vp = persist.tile([128, NKC, 65 * HPC], BF)
    ctn = persist.tile([128, 2, S], BF)
    ctu = persist.tile([65, 16, 512], BF)     # unnormalized ctx + L, per (h, qt)
    lpack = persist.tile([16, 8, 64], BF)     # packed L per strip: 2 heads x 8x64
    ldram = dpool.tile([8, 1024], BF)         # recips per strip, head-major

    # ---- input DMAs: what strip (0,0) needs comes first ----
    xr = xT.rearrange("(c p) s -> p c s", p=128)
    wqr = wqt.rearrange("(c p) j -> p c j", p=128)
    wkr = wkt.rearrange("(c p) j -> p c j", p=128)
    wvr = wvt.rearrange("(c p) j -> p c j", p=128)
    wor = wot.rearrange("(c p) o -> p c o", p=128)

    nc.sync.dma_start(out=pad_s, in_=pad0)
    nc.sync.dma_start(out=tri_s, in_=tri)
    # ones columns of vp: broadcast the bf16 pad column into all 4 head slots
    # on the vector engine (a strided 2-byte DMA would hog the ring for ~15us)
    padb_s = persist.tile([128, NKC], BF)
    nc.sync.dma_start(out=padb_s, in_=padb)
    vp_ones = vp.rearrange("p k (h u) -> p k h u", u=65)[:, :, :, 64:65]
    padb_b = bass.AP(
        tensor=padb_s.tensor, offset=padb_s.offset,
        ap=[list(padb_s.ap[0]), list(padb_s.ap[1]), [0, HPC], [0, 1]],
    )
    nc.vector.tensor_copy(out=vp_ones, in_=padb_b)

    # Input DMAs on sync/gpsimd/scalar (scalar's issues all land before the
    # first exp needs its queue).
    early = [nc.sync, nc.gpsimd, nc.scalar]
    ei = 0

    def dma_early(out, in_):
        nonlocal ei
        early[ei % len(early)].dma_start(out=out, in_=in_)
        ei += 1

    # strict need-order: K proj needs wk+xs-st0, then Q needs wq, then V
    # needs wv; the rest of xs round-robins over the same queues so no queue
    # races ahead and steals HBM bandwidth from the critical set
    for c in range(8):
        dma_early(wk_s[:, c, :], wkr[:, c, :])
        dma_early(xs[:, c, 0:512], xr[:, c, 0:512])
        ei += 1  # co-prime rotation: alternate which queue gets each type
    for c in range(8):
        dma_early(wq_s[:, c, :], wqr[:, c, :])
    for c in range(8):
        dma_early(wv_s[:, c, :], wvr[:, c, :])
    for st in range(1, 4):
        for c in range(8):
            dma_early(
                xs[:, c, 512 * st : 512 * st + 512],
                xr[:, c, 512 * st : 512 * st + 512],
            )
    for c in range(2):
        dma_early(wo_s[:, c, :], wor[:, c, :])

    # ---- emission helpers ----
    def qk_half(hp, wi, st, half, state):
        """Half (4 dc) of one Q/K [128, 512] s-tile projection for pair hp."""
        wsb, dst = ((wq_s, qt2), (wk_s, kt2))[wi]
        if half == 0:
            state["ps"] = pj_pool.tile(
                [128, 512], F32, tag="pj", name=f"qk{hp}{wi}{st}"
            )
        ps = state["ps"]
        for dc in range(4 * half, 4 * half + 4):
            nc.tensor.matmul(
                ps,
                wsb[:, dc, 128 * hp : 128 * hp + 128],
                xs[:, dc, 512 * st : 512 * st + 512],
                start=(dc == 0),
                stop=(dc == 7),
            )
        if half == 1:
            nc.vector.tensor_copy(
                out=dst[:, hp, 512 * st : 512 * st + 512], in_=ps
            )

    def qk_group(hp, wi, st):
        state = {}
        qk_half(hp, wi, st, 0, state)
        qk_half(hp, wi, st, 1, state)

    def v_half(sc, half, state):
        """Half (4 dc) of V key-chunk sc projection + padding mask on finish."""
        if half == 0:
            state["ps"] = pj_pool.tile([128, 512], F32, tag="pj", name=f"v{sc}")
        ps = state["ps"]
        for dc in range(4 * half, 4 * half + 4):
            nc.tensor.matmul(
                ps[:, 0:256],
                xs[:, dc, 128 * sc : 128 * sc + 128],
                wv_s[:, dc, :],
                start=(dc == 0),
                stop=(dc == 7),
            )
        if half == 1:
            dst = vp[:, sc, :].rearrange("p (h u) -> p h u", u=65)[:, :, 0:64]
            src = ps[:, 0:256].rearrange("p (h u) -> p h u", u=64)
            nc.vector.tensor_scalar_mul(dst, src, pad_s[:, sc : sc + 1])

    def v_group(sc):
        state = {}
        v_half(sc, 0, state)
        v_half(sc, 1, state)

    yr = yT.rearrange("(ot p) s -> ot p s", p=128)

    attn_done = [False]

    def out_unit(st, ot, tail):
        # post-attention, the sc psum banks are free: deepen the rotation so
        # MM pairs never wait on the staging copies (keeps HAM warm in the tail)
        if attn_done[0] and ot % 2 == 0:
            ps = sc_pool.tile([128, 1024], F32, tag="sc", name=f"o{st}{ot}")[:, 0:512]
        else:
            ps = pj_pool.tile([128, 512], F32, tag="pj", name=f"o{st}{ot}")
        for c2 in range(2):
            nc.tensor.matmul(
                ps,
                wo_s[:, c2, 128 * ot : 128 * ot + 128],
                ctn[:, c2, 512 * st : 512 * st + 512],
                start=(c2 == 0),
                stop=(c2 == 1),
            )
        ystg = work.tile([128, 512], BF, tag="y", name=f"ys{st}{ot}")
        if tail and ot % 2 == 0:
            nc.scalar.copy(ystg, ps)
        else:
            nc.vector.tensor_copy(out=ystg, in_=ps)
        dq = nc.gpsimd if ot % 2 == 0 else nc.sync
        dq.dma_start(out=yr[ot, :, 512 * st : 512 * st + 512], in_=ystg)

    # ---- filler scheduler ----
    filler: list = []   # (key, cost_ns, emit_fn)
    done_keys = set()

    def emit_next():
        key, cost, fn = filler.pop(0)
        fn()
        done_keys.add(key)
        return cost

    def need(keys):
        keys = [k for k in keys if k not in done_keys]
        forced = False
        while keys:
            emit_next()
            forced = True
            keys = [k for k in keys if k not in done_keys]
        if forced:
            budget[0] = 0.0

    budget = [0.0]

    def run_filler():
        while filler and budget[0] > 0:
            budget[0] -= emit_next()

    # ---- attention chunk stream (software-pipelined across strips) ----
    pending_norm: list = []   # deferred per-strip normalize closures

    def emit_scores(ch):
        hp, qt, kc, w, qs, si = ch["hp"], ch["qt"], ch["kc"], ch["w"], ch["qs"], ch["si"]
        K0 = 128 * kc
        sc = sc_pool.tile([128, 1024], F32, tag="sc", name=f"sc{si}_{kc}")
        nc.tensor.matmul(
            sc[:, 0 : w], kt2[0:64, hp, K0 : K0 + 128],
            qt2[0:64, hp, qs : qs + w],
            start=True, stop=True,
        )
        nc.tensor.matmul(
            sc[:, 512 : 512 + w], kt2[64:128, hp, K0 : K0 + 128],
            qt2[64:128, hp, qs : qs + w],
            start=True, stop=True,
        )
        pu = pu_pool.tile([128, 1024], BF, tag="pu", name=f"pu{si}_{kc}")
        sc2 = sc.rearrange("p (t f) -> p t f", t=2)[:, :, 0:w]
        pu2 = pu.rearrange("p (t f) -> p t f", t=2)[:, :, 0:w]
        nc.scalar.activation(out=pu2, in_=sc2, func=EXP, scale=0.125)
        if ch["band"]:
            tsl = tri_s[:, 384 : 384 + w]
            tri_b = bass.AP(
                tensor=tsl.tensor, offset=tsl.offset,
                ap=[list(tsl.ap[0]), [0, 2], list(tsl.ap[1])],
            )
            nc.vector.tensor_mul(pu2, pu2, tri_b)
        ch["pu"] = pu

    def emit_pv(ch):
        hp, kc, w, co = ch["hp"], ch["kc"], ch["w"], ch["co"]
        pu = ch["pu"]
        he, ho = 2 * hp, 2 * hp + 1
        ct_e, ct_o = ch["ct"]
        nc.tensor.matmul(
            ct_e[:, co : co + w],
            vp[:, kc, 65 * he : 65 * he + 65], pu[:, 0:w],
            start=(kc == 0), stop=ch["last"],
        )
        nc.tensor.matmul(
            ct_o[:, co : co + w],
            vp[:, kc, 65 * ho : 65 * ho + 65], pu[:, 512 : 512 + w],
            start=(kc == 0), stop=ch["last"],
        )

    def emit_post(ch):
        """After a strip's last PV: stage ctx+L, pack L, recip, bounce."""
        hp, qt, si = ch["hp"], ch["qt"], ch["si"]
        ct_e, ct_o = ch["ct"]
        for idx, cta in ((0, ct_e), (1, ct_o)):
            hq = (2 * hp + idx) * 4 + qt
            nc.vector.tensor_copy(out=ctu[:, hq, :], in_=cta)
            nc.sync.dma_start(
                out=lpack[8 * idx : 8 * idx + 8, si, :], in_=ctu[64:65, hq, :]
            )
        lp = lpack[:, si, :]
        nc.vector.tensor_scalar_max(lp, lp, 1e-30)
        with nc.allow_low_precision(reason="1/L in bf16: 0.4% rel, budget 2e-2"):
            nc.vector.reciprocal(lp, lp)
        nc.sync.dma_start(out=ldram[si : si + 1, :], in_=lp)

        Q0 = 512 * qt

        def norm_tail(hp=hp, qt=qt, si=si, Q0=Q0):
            for idx in (0, 1):
                hq = (2 * hp + idx) * 4 + qt
                rlb = work.tile([64, 512], BF, tag="rlb", name=f"rlb{si}{idx}")
                row = ldram[si : si + 1, 512 * idx : 512 * idx + 512]
                bsrc = bass.AP(
                    tensor=row.tensor, offset=row.offset,
                    ap=[[0, 64]] + list(row.ap[1:]),
                )
                nc.sync.dma_start(out=rlb, in_=bsrc)
                if idx == 0:
                    nc.vector.tensor_mul(
                        ctn[0:64, hp, Q0 : Q0 + 512], ctu[0:64, hq, :], rlb
                    )
                else:
                    stg = work.tile([64, 512], BF, tag="stg", name=f"stg{si}")
                    nc.vector.tensor_mul(stg, ctu[0:64, hq, :], rlb)
                    nc.sync.dma_start(
                        out=ctn[64:128, hp, Q0 : Q0 + 512], in_=stg
                    )

        pending_norm.append((hp, qt, norm_tail))

    # ---- build filler queue ----
    def add_qk(hp, wi, st):
        state = {}
        filler.append(
            (("qk2", hp, wi, st), 915.0,
             lambda hp=hp, wi=wi, st=st, state=state: qk_half(hp, wi, st, 0, state))
        )
        filler.append(
            (("qk", hp, wi, st), 915.0,
             lambda hp=hp, wi=wi, st=st, state=state: qk_half(hp, wi, st, 1, state))
        )

    def add_v(sc):
        state = {}
        filler.append(
            (("v2", sc), 750.0, lambda sc=sc, state=state: v_half(sc, 0, state))
        )
        filler.append(
            (("v", sc), 750.0, lambda sc=sc, state=state: v_half(sc, 1, state))
        )

    def add_out(st, tail=False):
        for ot in range(8):
            filler.append(
                (("out", st, ot), 480.0,
                 lambda st=st, ot=ot, tail=tail: out_unit(st, ot, tail))
            )

    for sc in range(1, 4):
        add_v(sc)
    add_qk(1, 1, 0)
    add_qk(1, 0, 0)
    add_qk(0, 1, 1)
    add_qk(0, 0, 1)
    for sc in range(4, 8):
        add_v(sc)
    add_qk(1, 1, 1)
    add_qk(1, 0, 1)
    add_qk(0, 1, 2)
    add_qk(0, 0, 2)
    for sc in range(8, 12):
        add_v(sc)
    add_qk(1, 1, 2)
    add_qk(1, 0, 2)
    add_qk(0, 1, 3)
    add_qk(0, 0, 3)
    for sc in range(12, 16):
        add_v(sc)
    add_qk(1, 1, 3)
    add_qk(1, 0, 3)

    # ---- main schedule ----
    # HAM warmup: ~3.4us of dummy matmuls on scratch while the input DMAs
    # land, so the real projections start at 2.4 GHz instead of 1.2
    nc.vector.memset(scratch, 0.0)
    du = pj_pool.tile([128, 512], F32, tag="pj", name="du")
    for _ in range(16):
        nc.tensor.matmul(
            du[:, 0:256], scratch[:, 0:128], scratch, start=True, stop=True
        )

    # pre-phase: the bare minimum for strip (0,0) chunk 0 — pair-0 st0
    # projections and V chunk 0. Everything else fills in mid-stream.
    qk_group(0, 1, 0)
    qk_group(0, 0, 0)
    v_group(0)
    done_keys.update({("qk", 0, 1, 0), ("qk", 0, 0, 0), ("v", 0)})

    def flush_norm():
        nhp, nqt, fn = pending_norm.pop(0)
        fn()
        if nhp == 1:
            add_out(nqt, tail=(nqt == 3))

    # flat chunk list over the interleaved strip order
    chunks = []
    for qt in range(4):
        for hp in range(2):
            si = 2 * qt + hp
            Q0 = 512 * qt
            nkc = 4 * qt + 4
            for kc in range(nkc):
                K0 = 128 * kc
                band = K0 >= Q0
                qs = K0 if band else Q0
                chunks.append({
                    "hp": hp, "qt": qt, "kc": kc, "si": si, "band": band,
                    "qs": qs, "w": Q0 + 512 - qs, "co": qs - Q0,
                    "first": kc == 0, "last": kc == nkc - 1,
                })

    prev = None
    for ch in chunks:
        if ch["first"]:
            need(
                [("qk", ch["hp"], 1, st) for st in range(ch["qt"] + 1)]
                + [("qk", ch["hp"], 0, ch["qt"])]
            )
            si = ch["si"]
            ct = (
                ct_pool.tile([65, 512], F32, tag="ct", name=f"cte{si}"),
                ct_pool.tile([65, 512], F32, tag="ct", name=f"cto{si}"),
            )
        ch["ct"] = ct
        emit_scores(ch)
        if prev is not None:
            budget[0] = min(
                budget[0]
                + (2 * prev["w"] + 352) / 1.2
                - (3 * prev["w"]) / 2.4,
                3000.0,
            )
            run_filler()
            # V chunks are forced just-in-time (one chunk ahead of their PV)
            need([("v", prev["kc"]), ("v", min(prev["kc"] + 1, NKC - 1))])
            emit_pv(prev)
            if prev["last"]:
                emit_post(prev)
                # flush the strip-before-last's normalize (its DRAM bounce
                # has had a full strip of time)
                if len(pending_norm) >= 2:
                    flush_norm()
        prev = ch
    need([("v", prev["kc"])])
    emit_pv(prev)
    emit_post(prev)
    attn_done[0] = True
    while pending_norm:
        flush_norm()
    # drain remaining filler (out_proj of late qts)
    while filler:
        emit_next()


def build_nc():
    nc = bacc.Bacc(
        "TRN2",
        target_bir_lowering=False,
        debug=False,
        enable_asserts=False,
        num_devices=NCORES,
    )
    from contextlib import ExitStack

    with tile.TileContext(nc) as tc:
        with ExitStack() as ctx:
            _emit(tc, ctx)
    nc.compile()
    return nc


def _get_nc():
    if not _NC_CACHE:
        _NC_CACHE.append(build_nc())
    return _NC_CACHE[0]


def make_tri() -> np.ndarray:
    p = np.arange(128)[:, None]
    v = np.arange(TRI_W)[None, :]
    return (p <= v - 384).astype(np.float32).astype(ml_dtypes.bfloat16)


def make_in_maps(x, mask, WQ, WK, WV, WO):
    bf = ml_dtypes.bfloat16
    tri = make_tri()
    in_maps = []
    for c in range(NCORES):
        b, g = c // (NCORES // B), c % (NCORES // B)
        sl = slice(DSH * g, DSH * g + DSH)
        pad = np.ascontiguousarray(
            (mask[b] == 0).astype(np.float32).reshape(NKC, 128).T
        )
        in_maps.append(
            {
                "xT": np.ascontiguousarray(x[b].T).astype(bf),
                "wqt": np.ascontiguousarray(WQ[sl, :].T).astype(bf),
                "wkt": np.ascontiguousarray(WK[sl, :].T).astype(bf),
                "wvt": np.ascontiguousarray(WV[sl, :].T).astype(bf),
                "wot": np.ascontiguousarray(WO[:, sl].T).astype(bf),
                "pad0": pad,
                "padb": pad.astype(bf),
                "tri": tri,
            }
        )
    return in_maps


def assemble(results, x, mask, WV, WO, bO) -> np.ndarray:
    y = np.zeros((B, S, D), np.float32)
    for c in range(NCORES):
        y[c // (NCORES // B)] += results[c]["yT"].T
    y += bO[None, None, :]
    # Rows i < first-unmasked-index are fully masked in the reference; its
    # softmax then degenerates to uniform attention over all positions.
    for b in range(B):
        nz = np.nonzero(mask[b] == 0)[0]
        t = int(nz[0]) if nz.size else S
        if t > 0:
            vbar = x[b].mean(axis=0) @ WV.T
            yfix = vbar @ WO.T + bO
            y[b, :t, :] = yfix
    return y


def kernel(x, mask, WQ, WK, WV, WO, bO) -> np.ndarray:
    x = np.asarray(x, np.float32)
    mask = np.asarray(mask, np.int32)
    WQ = np.asarray(WQ, np.float32)
    WK = np.asarray(WK, np.float32)
    WV = np.asarray(WV, np.float32)
    WO = np.asarray(WO, np.float32)
    bO = np.asarray(bO, np.float32)

    nc = _get_nc()
    in_maps = make_in_maps(x, mask, WQ, WK, WV, WO)
    res = run_bass_kernel_spmd(nc, in_maps, list(range(NCORES)))
    return assemble(res.results, x, mask, WV, WO, bO)
